# revision 30
# baseline (speedup 1.0000x reference)
"""Multi-head attention Trainium2 Bass kernel (v2).

Problem: B=2, T=2048, D=1024, H=16 heads, dk=64 (fp32).
  out = softmax((x@Wq.T+bq)(x@Wk.T+bk).T / 8) (x@Wv.T+bv) @ Wo.T + bo

Sharding (8 cores): data-parallel over B (2) x tensor-parallel over 4
head-groups of 4 heads.  Core (b, g) computes, for batch b and heads
[4g, 4g+4): Q/K/V projections (column-sliced Wq/Wk/Wv), attention, and
the row-sliced Wo projection, producing a partial (2048, 1024) fp16
output.  Host sums the partials per batch in fp32 and adds the bias
terms.

Bias algebra (removes all device-side bias work except bq):
  - bk shifts every score of a query by a constant -> softmax-invariant
    -> dropped entirely.
  - bv: softmax rows sum to 1, so the bv contribution to the output is
    the constant row bv @ Wo.T -> folded into bo on the host.
  - bq: added on the Q-projection eviction via a per-partition
    tensor_scalar add (Q.T layout has features on partitions).

Per-core device schedule (everything fp16 operands, fp32 PSUM):
  - One persistent PSUM tensor sf [128, 4096] (all 8 banks) managed
    manually with subtile dependency tracking - no pool barriers, so
    the scheduler freely overlaps phases.
  - Projections (k-outer, 8 full-bank chains): K.T -> V -> Q.T, each
    chain accumulates 8 k-tiles; DMAs are issued in consumption order
    so the PE starts as soon as wk0+xt0 land.
  - V stored as V_aug [128, 16*384]: per key-tile, per head-pair block
    [V_even|ones64|V_odd] so the PV matmul also produces the softmax
    denominator (replicated across 64 partitions) for free.
  - Attention per (chunk c of 512 queries, head-pair):  scores.T tiles
    [128 keys, 512q] per head, both heads of the pair packed into one
    1024-wide PSUM slot (row-group-concurrent matmuls, contraction 64).
    3 slots (banks 0-5) rotate; ScalarE exp's TWO slots per ACTIVATE
    (2048 wide, via a 3D AP, negative-stride for the wrap pattern) to
    amortize the ~313-cycle ACT overhead.  PV accumulates in banks 6-7.
  - Normalization: denominators evicted to fp16 SBUF; 1/d via int16
    magic-subtract seed + one fp16 Newton step (beats the DVE's 8
    cycle/element iterative reciprocal ~3x); O * (1/d) in fp16.
  - Output projection accumulates head-pairs in banks 6/7 (after PV is
    evicted), evicts fp16, DMAs fp16 partials out (halves DMA bytes).
"""

import numpy as np

D = 1024          # d_model
T = 2048          # sequence length
G = 256           # features per head-group (4 heads * 64)
DK = 64
NKT = D // 128    # 8 contraction tiles for projections
NTT = T // 128    # 16 key tiles
NCH = T // 512    # 4 query chunks of 512
VROW = 2 * 192    # V_aug row per key tile: 2 blocks of [V_e|ones64|V_o]
MAGIC = 0x7798    # fp16 reciprocal seed: bitcast(MAGIC - bits16(d))
# fp16 Schraudolph exp for the DVE half: bitcast16(rint(s*EXP_A + EXP_B))
# ~= exp(s/8), max rel err ~3% pointwise, ~6.5e-3 end-to-end (softmax
# weights are consistent: the denominator sums the same approximated p).
EXP_A = 0.125 * 1.4426950408889634 * 1024.0
EXP_B = 15360.0 - 44.5

_CACHE = {}


def _split_multi_waits(nc):
    """walrus's TRN2 codegen rejects >1 sync-wait on datapath instruction
    structs.  Hoist every wait of a multi-wait datapath instruction onto
    single-wait NoOps just before it on the same engine queue."""
    import concourse.mybir as mybir

    keep = ("InstEventSemaphore", "InstUnconditionalBranch",
            "InstCall", "InstBranchHint", "InstHalt", "InstNoOp",
            "InstAllEngineBarrier", "InstCompareAndBranch")
    nid = [0]
    for f in nc.m.functions:
        for bb in f.blocks:
            new = []
            for ins in bb.instructions:
                si = ins.sync_info
                waits = list(si.on_wait) if si and si.on_wait else []
                if len(waits) >= 2 and type(ins).__name__ not in keep:
                    for w in waits:
                        nid[0] += 1
                        nop = mybir.InstNoOp(name=f"{ins.name}-wsplit{nid[0]}",
                                             ins=[], outs=[])
                        nop.engine = ins.engine
                        nop.sync_info = mybir.SyncInfo(on_wait=[w], on_update=[])
                        new.append(nop)
                    ins.sync_info = mybir.SyncInfo(
                        on_wait=[], on_update=list(si.on_update or []))
                new.append(ins)
            bb.instructions = new
    return nc


def _build(split_waits=True):
    import concourse.bass as bass
    import concourse.mybir as mybir
    import concourse.tile as tile

    f32 = mybir.dt.float32
    f16 = mybir.dt.float16
    i16 = mybir.dt.int16
    ALU = mybir.AluOpType
    EXP = mybir.ActivationFunctionType.Exp
    CPY = mybir.ActivationFunctionType.Copy
    nc = bass.Bass()

    xT = nc.dram_tensor("xT", [D, T], f16, kind="ExternalInput")
    wqT = nc.dram_tensor("wqT", [D, G], f16, kind="ExternalInput")
    wkT = nc.dram_tensor("wkT", [D, G], f16, kind="ExternalInput")
    wvT = nc.dram_tensor("wvT", [D, G], f16, kind="ExternalInput")
    woT = nc.dram_tensor("woT", [G, D], f16, kind="ExternalInput")
    bqc = nc.dram_tensor("bqc", [128, 2], f32, kind="ExternalInput")
    out = nc.dram_tensor("out", [T, D], f16, kind="ExternalOutput")

    with tile.TileContext(nc) as tc:
        with tc.tile_pool(name="sb", bufs=1) as sb, \
             tc.tile_pool(name="dyn", bufs=2) as dyn, \
             tc.tile_pool(name="ps", bufs=1, space="PSUM") as ps:

            # ---- DMAs in consumption order ----
            wk_sb, xt = [], []
            for k in range(NKT):
                t = sb.tile([128, G], f16, tag=f"wk{k}", name=f"wk{k}")
                nc.sync.dma_start(out=t, in_=wkT[k * 128:(k + 1) * 128, :])
                wk_sb.append(t)
                t = sb.tile([128, T], f16, tag=f"xt{k}", name=f"xt{k}")
                nc.sync.dma_start(out=t, in_=xT[k * 128:(k + 1) * 128, :])
                xt.append(t)
            bq_sb = sb.tile([128, 2], f32, tag="bq", name="bq_sb")
            nc.sync.dma_start(out=bq_sb, in_=bqc[:, :])
            # warm the ScalarE exp table-set (~2.7us) during the DMA wait
            scr = sb.tile([128, 2], f16, tag="scr", name="scr")
            nc.scalar.activation(out=scr, in_=bq_sb, func=EXP, scale=0.0)
            wv_sb, wq_sb = [], []
            for nm, dram, lst in (("wv", wvT, wv_sb), ("wq", wqT, wq_sb)):
                for k in range(NKT):
                    t = sb.tile([128, G], f16, tag=f"{nm}{k}", name=f"{nm}{k}")
                    nc.sync.dma_start(out=t, in_=dram[k * 128:(k + 1) * 128, :])
                    lst.append(t)
            wo_sb = []
            for p2 in range(2):
                t = sb.tile([128, D], f16, tag=f"wo{p2}", name=f"wo{p2}")
                nc.sync.dma_start(out=t, in_=woT[p2 * 128:(p2 + 1) * 128, :])
                wo_sb.append(t)

            # ---- persistent SBUF ----
            qt = [sb.tile([128, T], f16, tag=f"qt{p}", name=f"qt{p}")
                  for p in range(2)]
            kt = [sb.tile([128, T], f16, tag=f"kt{p}", name=f"kt{p}")
                  for p in range(2)]
            va = sb.tile([128, NTT * VROW], f16, tag="va", name="va")
            va6 = va.rearrange("p (t b x) -> p t b x", t=NTT, b=6)
            nc.vector.memset(va6[:, :, 1::3, :], 1.0)   # ones64 columns

            # ---- the one PSUM tensor: 8 banks, manual ranges ----
            sf = ps.tile([128, 4096], f32, tag="sf", name="sf")
            sf3 = sf[:, 0:3072].rearrange("p (s x) -> p s x", s=3)
            pv_e = sf[:, 3072:3584]
            pv_o = sf[:, 3584:4096]

            def chain(i):       # 8 full-bank projection chains
                return sf[:, i * 512:(i + 1) * 512]

            # ---- K.T projection: chains (p2, c), k-outer ----
            for k in range(NKT):
                for i in range(8):
                    p2, c = divmod(i, 4)
                    nc.tensor.matmul(
                        out=chain(i),
                        lhsT=wk_sb[k][:, p2 * 128:(p2 + 1) * 128],
                        rhs=xt[k][:, c * 512:(c + 1) * 512],
                        start=(k == 0), stop=(k == NKT - 1))
            for i in range(8):
                p2, c = divmod(i, 4)
                nc.vector.tensor_copy(
                    out=kt[p2][:, c * 512:(c + 1) * 512], in_=chain(i))

            # ---- V projection: chain-major so V tiles complete (and are
            # evicted) progressively; bank ring tt%8 handles wave reuse ----
            va5 = va.rearrange("p (t pr b x) -> p t pr b x", t=NTT, pr=2, b=3)
            for tt in range(NTT):
                base = (tt % 8) * 512
                for k in range(NKT):
                    nc.tensor.matmul(
                        out=sf[:, base:base + G],
                        lhsT=xt[k][:, tt * 128:(tt + 1) * 128],
                        rhs=wv_sb[k][:, :],
                        start=(k == 0), stop=(k == NKT - 1))
                nc.vector.tensor_copy(
                    out=va5[:, tt, :, 0::2, :],
                    in_=sf[:, base:base + 256].rearrange(
                        "p (pr h x) -> p pr h x", pr=2, h=2))

            # ---- Q.T projection (+bq on eviction), chain-major, chunk-0
            # chains first so stage B's first scores unblock earliest ----
            for i, (c, p2) in enumerate((c, p2) for c in range(4)
                                        for p2 in range(2)):
                for k in range(NKT):
                    nc.tensor.matmul(
                        out=chain(i),
                        lhsT=wq_sb[k][:, p2 * 128:(p2 + 1) * 128],
                        rhs=xt[k][:, c * 512:(c + 1) * 512],
                        start=(k == 0), stop=(k == NKT - 1))
                nc.vector.tensor_scalar(
                    out=qt[p2][:, c * 512:(c + 1) * 512], in0=chain(i),
                    scalar1=bq_sb[:, p2:p2 + 1], scalar2=None, op0=ALU.add)

            # ---- attention + output projection ----
            # Flat software pipeline at KEY-TILE granularity (128 tiles).
            # Per-engine queues are strict FIFO, so emission order IS the
            # schedule skeleton: scores run TWO tiles ahead of pv so the
            # exp streams never wait on the PE queue head; exp alternates
            # engines per tile (even: exact ScalarE, odd: DVE fast-exp) so
            # both stream back-to-back on disjoint PSUM banks; O-proj is
            # emitted a few tiles into the next chunk so the DVE
            # normalization latency is hidden.
            tk_units = [(c, pair, tkl) for c in range(NCH)
                        for pair in range(2) for tkl in range(NTT)]
            onorm = {}          # (c, pair) -> normalized O tile

            def emit_scores(t):
                c, pair, tkl = tk_units[t]
                cs = slice(c * 512, (c + 1) * 512)
                s = t % 3
                for h in range(2):   # packed row-group pair
                    nc.tensor.matmul(
                        out=sf[:, s * 1024 + h * 512:
                               s * 1024 + (h + 1) * 512],
                        lhsT=kt[pair][h * 64:(h + 1) * 64,
                                      tkl * 128:(tkl + 1) * 128],
                        rhs=qt[pair][h * 64:(h + 1) * 64, cs],
                        start=True, stop=True)

            def emit_exp(t):
                c, pair, tkl = tk_units[t]
                s = t % 3
                if t % 2 == 0:
                    pa = dyn.tile([128, 1024], f16, tag="pa", bufs=7,
                                  name=f"pa_{c}_{pair}_{tkl}")
                    nc.scalar.activation(out=pa, in_=sf3[:, s, :], func=EXP,
                                         scale=0.125)
                    return pa
                pb = dyn.tile([128, 1024], i16, tag="pb", bufs=7,
                              name=f"pb_{c}_{pair}_{tkl}")
                nc.vector.tensor_scalar(
                    out=pb, in0=sf3[:, s, :], scalar1=EXP_A, scalar2=EXP_B,
                    op0=ALU.mult, op1=ALU.add)
                return pb.bitcast(f16)

            def emit_pv(t, p):
                c, pair, tkl = tk_units[t]
                off = tkl * VROW + pair * 192
                nc.tensor.matmul(
                    out=pv_e, lhsT=va[:, off:off + 128], rhs=p[:, 0:512],
                    start=(tkl == 0), stop=(tkl == NTT - 1))
                nc.tensor.matmul(
                    out=pv_o, lhsT=va[:, off + 64:off + 192],
                    rhs=p[:, 512:1024],
                    start=(tkl == 0), stop=(tkl == NTT - 1))

            def emit_norm(c, pair):
                # pv_e = [O_e; d_e], pv_o = [d_o; O_o]; 1/d via int16
                # magic seed + one fp16 Newton step.  Work is spread:
                # base-aligned O evicts + the magic subtract on ScalarE,
                # cross-base denominator evicts on the DVE.  The Newton
                # chain for pair 1 gates the chunk's O-projection, so it
                # runs on the fast DVE; pair 0's (latency-insensitive)
                # runs on the otherwise-idle GpSimd.
                eng = nc.vector if pair == 1 else nc.gpsimd
                oo = dyn.tile([128, 512], f16, tag="oo", name=f"oo{c}{pair}")
                dd = dyn.tile([128, 512], f16, tag="dd", name=f"dd{c}{pair}")
                nc.scalar.activation(out=oo[0:64, :], in_=pv_e[0:64, :],
                                     func=CPY)
                nc.scalar.activation(out=oo[64:128, :], in_=pv_o[64:128, :],
                                     func=CPY)
                nc.vector.tensor_copy(out=dd[0:64, :], in_=pv_e[64:128, :])
                nc.vector.tensor_copy(out=dd[64:128, :], in_=pv_o[0:64, :])
                r0 = dyn.tile([128, 512], i16, tag="r0", name=f"r0{c}{pair}")
                nc.scalar.activation(out=r0, in_=dd.bitcast(i16), func=CPY,
                                     scale=-1.0, bias=float(MAGIC))
                r = r0.bitcast(f16)
                tn = dyn.tile([128, 512], f16, tag="tn", name=f"tn{c}{pair}")
                eng.tensor_tensor(out=tn, in0=dd, in1=r, op=ALU.mult)
                un = dyn.tile([128, 512], f16, tag="un", name=f"un{c}{pair}")
                eng.tensor_scalar(
                    out=un, in0=tn, scalar1=-1.0, scalar2=2.0,
                    op0=ALU.mult, op1=ALU.add)
                r1 = dyn.tile([128, 512], f16, tag="r1", name=f"r1{c}{pair}")
                eng.tensor_tensor(out=r1, in0=r, in1=un, op=ALU.mult)
                on = dyn.tile([128, 512], f16, tag=f"on{pair}",
                              name=f"on{c}{pair}")
                eng.tensor_tensor(out=on, in0=oo, in1=r1, op=ALU.mult)
                onorm[(c, pair)] = on

            def emit_oproj(c):
                for mt in range(4):
                    for n2 in range(2):
                        j = mt * 2 + n2
                        ops = sf[:, 3072 + (j % 2) * 512:
                                 3072 + (j % 2) * 512 + 512]
                        for pair in range(2):
                            nc.tensor.matmul(
                                out=ops,
                                lhsT=onorm[(c, pair)][:, mt * 128:(mt + 1) * 128],
                                rhs=wo_sb[pair][:, n2 * 512:(n2 + 1) * 512],
                                start=(pair == 0), stop=(pair == 1))
                        osb = dyn.tile([128, 512], f16, tag="osb", bufs=4,
                                       name=f"osb_{c}_{mt}_{n2}")
                        nc.vector.tensor_copy(out=osb, in_=ops)
                        nc.sync.dma_start(
                            out=out[c * 512 + mt * 128:c * 512 + (mt + 1) * 128,
                                    n2 * 512:(n2 + 1) * 512],
                            in_=osb)

            # Emission = per-engine FIFO order.  Skews:
            #  - scores(t+2) before pv(t), so the PE always has the next
            #    exp's input written before either exp engine needs it.
            #  - normalization skewed 2 tiles late so its ScalarE/DVE ops
            #    never block the exp streams at the queue head.
            #  - O-proj(c) occupies banks 6/7 after chunk c's PV; the
            #    first 8 PV tiles of chunk c+1 (same banks) are held back
            #    until O-proj(c) is emitted.
            NT = len(tk_units)
            pas = {}
            pv_hold = []
            norm_due = None
            norm_wait = 0
            emit_scores(0)
            emit_scores(1)
            pas[0] = emit_exp(0)
            for t in range(NT):
                if t + 2 < NT:
                    emit_scores(t + 2)
                if t + 1 < NT:
                    pas[t + 1] = emit_exp(t + 1)
                if norm_due is not None and norm_wait == 0:
                    emit_norm(*norm_due)
                    norm_due = None
                norm_wait = max(0, norm_wait - 1)
                c, pair, tkl = tk_units[t]
                in_chunk = t % 32
                if c > 0 and in_chunk < 8:
                    pv_hold.append(t)
                    if in_chunk == 7:
                        emit_oproj(c - 1)
                        for th in pv_hold:
                            emit_pv(th, pas.pop(th))
                        pv_hold = []
                else:
                    emit_pv(t, pas.pop(t))
                if tkl == NTT - 1:
                    # flush happens next iteration, after that unit's
                    # scores/exp emission but BEFORE its pv (which reopens
                    # the pv banks with start=True)
                    norm_due = (c, pair)
                    norm_wait = 0
            emit_norm(*norm_due)
            emit_oproj(NCH - 1)
    if split_waits:
        _split_multi_waits(nc)
    return nc


def _get_nc(split_waits=True):
    key = ("nc", split_waits)
    if key not in _CACHE:
        _CACHE[key] = _build(split_waits)
    return _CACHE[key]


def make_in_maps(x, Wq, bq, Wk, bk, Wv, bv, Wo):
    dt = np.float16
    in_maps = []
    for core in range(8):
        b, g = divmod(core, 4)
        gs = slice(g * G, (g + 1) * G)
        in_maps.append({
            "xT": np.ascontiguousarray(x[b].T).astype(dt),
            "wqT": np.ascontiguousarray(Wq[gs, :].T).astype(dt),
            "wkT": np.ascontiguousarray(Wk[gs, :].T).astype(dt),
            "wvT": np.ascontiguousarray(Wv[gs, :].T).astype(dt),
            "woT": np.ascontiguousarray(Wo[:, gs].T).astype(dt),
            "bqc": np.ascontiguousarray(bq[gs].reshape(2, 128).T).astype(np.float32),
        })
    return in_maps


def host_out_init(bo, bv, Wo):
    """bo + bv @ Wo.T (the bv contribution is exact: softmax rows sum to 1)."""
    return (bo.astype(np.float64)
            + bv.astype(np.float64) @ Wo.T.astype(np.float64)).astype(np.float32)


def kernel(x, Wq, bq, Wk, bk, Wv, bv, Wo, bo):
    from concourse.bass_utils import run_bass_kernel_spmd

    x = np.asarray(x, dtype=np.float32)
    Wq = np.asarray(Wq, dtype=np.float32)
    Wk = np.asarray(Wk, dtype=np.float32)
    Wv = np.asarray(Wv, dtype=np.float32)
    Wo = np.asarray(Wo, dtype=np.float32)
    bq = np.asarray(bq, dtype=np.float32)
    bv = np.asarray(bv, dtype=np.float32)
    bo = np.asarray(bo, dtype=np.float32)

    nc = _get_nc()
    in_maps = make_in_maps(x, Wq, bq, Wk, bk, Wv, bv, Wo)

    res = run_bass_kernel_spmd(nc, in_maps, core_ids=list(range(8)))
    outp = np.tile(host_out_init(bo, bv, Wo)[None, None, :], (2, T, 1))
    for core in range(8):
        outp[core // 4] += res.results[core]["out"].astype(np.float32)
    return outp


# revision 32
# speedup vs baseline: 1.0180x; 1.0180x over previous
"""Multi-head attention Trainium2 Bass kernel (v2).

Problem: B=2, T=2048, D=1024, H=16 heads, dk=64 (fp32).
  out = softmax((x@Wq.T+bq)(x@Wk.T+bk).T / 8) (x@Wv.T+bv) @ Wo.T + bo

Sharding (8 cores): data-parallel over B (2) x tensor-parallel over 4
head-groups of 4 heads.  Core (b, g) computes, for batch b and heads
[4g, 4g+4): Q/K/V projections (column-sliced Wq/Wk/Wv), attention, and
the row-sliced Wo projection, producing a partial (2048, 1024) fp16
output.  Host sums the partials per batch in fp32 and adds the bias
terms.

Bias algebra (removes all device-side bias work except bq):
  - bk shifts every score of a query by a constant -> softmax-invariant
    -> dropped entirely.
  - bv: softmax rows sum to 1, so the bv contribution to the output is
    the constant row bv @ Wo.T -> folded into bo on the host.
  - bq: added on the Q-projection eviction via a per-partition
    tensor_scalar add (Q.T layout has features on partitions).

Per-core device schedule (everything fp16 operands, fp32 PSUM):
  - One persistent PSUM tensor sf [128, 4096] (all 8 banks) managed
    manually with subtile dependency tracking - no pool barriers, so
    the scheduler freely overlaps phases.
  - Projections (k-outer, 8 full-bank chains): K.T -> V -> Q.T, each
    chain accumulates 8 k-tiles; DMAs are issued in consumption order
    so the PE starts as soon as wk0+xt0 land.
  - V stored as V_aug [128, 16*384]: per key-tile, per head-pair block
    [V_even|ones64|V_odd] so the PV matmul also produces the softmax
    denominator (replicated across 64 partitions) for free.
  - Attention per (chunk c of 512 queries, head-pair):  scores.T tiles
    [128 keys, 512q] per head, both heads of the pair packed into one
    1024-wide PSUM slot (row-group-concurrent matmuls, contraction 64).
    3 slots (banks 0-5) rotate; ScalarE exp's TWO slots per ACTIVATE
    (2048 wide, via a 3D AP, negative-stride for the wrap pattern) to
    amortize the ~313-cycle ACT overhead.  PV accumulates in banks 6-7.
  - Normalization: denominators evicted to fp16 SBUF; 1/d via int16
    magic-subtract seed + one fp16 Newton step (beats the DVE's 8
    cycle/element iterative reciprocal ~3x); O * (1/d) in fp16.
  - Output projection accumulates head-pairs in banks 6/7 (after PV is
    evicted), evicts fp16, DMAs fp16 partials out (halves DMA bytes).
"""

import numpy as np

D = 1024          # d_model
T = 2048          # sequence length
G = 256           # features per head-group (4 heads * 64)
DK = 64
NKT = D // 128    # 8 contraction tiles for projections
NTT = T // 128    # 16 key tiles
NCH = T // 512    # 4 query chunks of 512
VROW = 2 * 192    # V_aug row per key tile: 2 blocks of [V_e|ones64|V_o]
MAGIC = 0x7798    # fp16 reciprocal seed: bitcast(MAGIC - bits16(d))
# fp16 Schraudolph exp for the DVE half: bitcast16(rint(s*EXP_A + EXP_B))
# ~= exp(s/8), max rel err ~3% pointwise, ~6.5e-3 end-to-end (softmax
# weights are consistent: the denominator sums the same approximated p).
EXP_A = 0.125 * 1.4426950408889634 * 1024.0
EXP_B = 15360.0 - 44.5

_CACHE = {}


def _split_multi_waits(nc):
    """walrus's TRN2 codegen rejects >1 sync-wait on datapath instruction
    structs.  Hoist every wait of a multi-wait datapath instruction onto
    single-wait NoOps just before it on the same engine queue."""
    import concourse.mybir as mybir

    keep = ("InstEventSemaphore", "InstUnconditionalBranch",
            "InstCall", "InstBranchHint", "InstHalt", "InstNoOp",
            "InstAllEngineBarrier", "InstCompareAndBranch")
    nid = [0]
    for f in nc.m.functions:
        for bb in f.blocks:
            new = []
            for ins in bb.instructions:
                si = ins.sync_info
                waits = list(si.on_wait) if si and si.on_wait else []
                if len(waits) >= 2 and type(ins).__name__ not in keep:
                    for w in waits:
                        nid[0] += 1
                        nop = mybir.InstNoOp(name=f"{ins.name}-wsplit{nid[0]}",
                                             ins=[], outs=[])
                        nop.engine = ins.engine
                        nop.sync_info = mybir.SyncInfo(on_wait=[w], on_update=[])
                        new.append(nop)
                    ins.sync_info = mybir.SyncInfo(
                        on_wait=[], on_update=list(si.on_update or []))
                new.append(ins)
            bb.instructions = new
    return nc


def _build(split_waits=True):
    import concourse.bass as bass
    import concourse.mybir as mybir
    import concourse.tile as tile

    f32 = mybir.dt.float32
    f16 = mybir.dt.float16
    i16 = mybir.dt.int16
    ALU = mybir.AluOpType
    EXP = mybir.ActivationFunctionType.Exp
    CPY = mybir.ActivationFunctionType.Copy
    nc = bass.Bass()

    xT = nc.dram_tensor("xT", [D, T], f16, kind="ExternalInput")
    wqT = nc.dram_tensor("wqT", [D, G], f16, kind="ExternalInput")
    wkT = nc.dram_tensor("wkT", [D, G], f16, kind="ExternalInput")
    wvT = nc.dram_tensor("wvT", [D, G], f16, kind="ExternalInput")
    woT = nc.dram_tensor("woT", [G, D], f16, kind="ExternalInput")
    bqc = nc.dram_tensor("bqc", [128, 2], f32, kind="ExternalInput")
    out = nc.dram_tensor("out", [T, D], f16, kind="ExternalOutput")

    with tile.TileContext(nc) as tc:
        with tc.tile_pool(name="sb", bufs=1) as sb, \
             tc.tile_pool(name="dyn", bufs=2) as dyn, \
             tc.tile_pool(name="ps", bufs=1, space="PSUM") as ps:

            # ---- DMAs in consumption order ----
            wk_sb, xt = [], []
            for k in range(NKT):
                t = sb.tile([128, G], f16, tag=f"wk{k}", name=f"wk{k}")
                nc.sync.dma_start(out=t, in_=wkT[k * 128:(k + 1) * 128, :])
                wk_sb.append(t)
                t = sb.tile([128, T], f16, tag=f"xt{k}", name=f"xt{k}")
                nc.sync.dma_start(out=t, in_=xT[k * 128:(k + 1) * 128, :])
                xt.append(t)
            bq_sb = sb.tile([128, 2], f32, tag="bq", name="bq_sb")
            nc.sync.dma_start(out=bq_sb, in_=bqc[:, :])
            # warm the ScalarE exp table-set (~2.7us) during the DMA wait
            scr = sb.tile([128, 2], f16, tag="scr", name="scr")
            nc.scalar.activation(out=scr, in_=bq_sb, func=EXP, scale=0.0)
            wv_sb, wq_sb = [], []
            for nm, dram, lst in (("wv", wvT, wv_sb), ("wq", wqT, wq_sb)):
                for k in range(NKT):
                    t = sb.tile([128, G], f16, tag=f"{nm}{k}", name=f"{nm}{k}")
                    nc.sync.dma_start(out=t, in_=dram[k * 128:(k + 1) * 128, :])
                    lst.append(t)
            wo_sb = []
            for p2 in range(2):
                t = sb.tile([128, D], f16, tag=f"wo{p2}", name=f"wo{p2}")
                nc.sync.dma_start(out=t, in_=woT[p2 * 128:(p2 + 1) * 128, :])
                wo_sb.append(t)

            # ---- persistent SBUF ----
            qt = [sb.tile([128, T], f16, tag=f"qt{p}", name=f"qt{p}")
                  for p in range(2)]
            kt = [sb.tile([128, T], f16, tag=f"kt{p}", name=f"kt{p}")
                  for p in range(2)]
            va = sb.tile([128, NTT * VROW], f16, tag="va", name="va")
            va6 = va.rearrange("p (t b x) -> p t b x", t=NTT, b=6)
            nc.vector.memset(va6[:, :, 1::3, :], 1.0)   # ones64 columns

            # ---- the one PSUM tensor: 8 banks, manual ranges ----
            sf = ps.tile([128, 4096], f32, tag="sf", name="sf")
            sf3 = sf[:, 0:3072].rearrange("p (s x) -> p s x", s=3)
            pv_e = sf[:, 3072:3584]
            pv_o = sf[:, 3584:4096]

            def chain(i):       # 8 full-bank projection chains
                return sf[:, i * 512:(i + 1) * 512]

            # ---- K.T projection: chains (p2, c), k-outer ----
            for k in range(NKT):
                for i in range(8):
                    p2, c = divmod(i, 4)
                    nc.tensor.matmul(
                        out=chain(i),
                        lhsT=wk_sb[k][:, p2 * 128:(p2 + 1) * 128],
                        rhs=xt[k][:, c * 512:(c + 1) * 512],
                        start=(k == 0), stop=(k == NKT - 1))
            for i in range(8):
                p2, c = divmod(i, 4)
                nc.vector.tensor_copy(
                    out=kt[p2][:, c * 512:(c + 1) * 512], in_=chain(i))

            # ---- V projection: chain-major so V tiles complete (and are
            # evicted) progressively; bank ring tt%8 handles wave reuse ----
            va5 = va.rearrange("p (t pr b x) -> p t pr b x", t=NTT, pr=2, b=3)
            for tt in range(NTT):
                base = (tt % 8) * 512
                for k in range(NKT):
                    nc.tensor.matmul(
                        out=sf[:, base:base + G],
                        lhsT=xt[k][:, tt * 128:(tt + 1) * 128],
                        rhs=wv_sb[k][:, :],
                        start=(k == 0), stop=(k == NKT - 1))
                nc.vector.tensor_copy(
                    out=va5[:, tt, :, 0::2, :],
                    in_=sf[:, base:base + 256].rearrange(
                        "p (pr h x) -> p pr h x", pr=2, h=2))

            # ---- Q.T projection (+bq on eviction), chain-major, chunk-0
            # chains first so stage B's first scores unblock earliest ----
            for i, (c, p2) in enumerate((c, p2) for c in range(4)
                                        for p2 in range(2)):
                for k in range(NKT):
                    nc.tensor.matmul(
                        out=chain(i),
                        lhsT=wq_sb[k][:, p2 * 128:(p2 + 1) * 128],
                        rhs=xt[k][:, c * 512:(c + 1) * 512],
                        start=(k == 0), stop=(k == NKT - 1))
                nc.vector.tensor_scalar(
                    out=qt[p2][:, c * 512:(c + 1) * 512], in0=chain(i),
                    scalar1=bq_sb[:, p2:p2 + 1], scalar2=None, op0=ALU.add)

            # ---- attention + output projection ----
            # Flat software pipeline over 64 "units" (one unit = 2 key
            # tiles of one (chunk, head-pair)).  Per-engine queues are
            # strict FIFO, so emission order IS the schedule skeleton:
            # scores(u+1) must be emitted BEFORE pv(u) (which waits on
            # exp(u)) or the PE queue head blocks and the whole loop
            # serializes; O-proj is emitted 2 units after its chunk ends
            # so the DVE normalization latency is hidden.
            units = [(c, pair, g) for c in range(NCH) for pair in range(2)
                     for g in range(8)]
            onorm = {}          # (c, pair) -> normalized O tile

            def emit_scores(u):
                c, pair, g = units[u]
                cs = slice(c * 512, (c + 1) * 512)
                sA, sB = (2 * u) % 3, (2 * u + 1) % 3
                for tk, s in ((2 * g, sA), (2 * g + 1, sB)):
                    for h in range(2):   # packed row-group pair
                        nc.tensor.matmul(
                            out=sf[:, s * 1024 + h * 512:
                                   s * 1024 + (h + 1) * 512],
                            lhsT=kt[pair][h * 64:(h + 1) * 64,
                                          tk * 128:(tk + 1) * 128],
                            rhs=qt[pair][h * 64:(h + 1) * 64, cs],
                            start=True, stop=True)

            def emit_exp(u):
                # tkA: exact exp on ScalarE; tkB: Schraudolph fast-exp on
                # the DVE (one fused mul-add into int16, bitcast to fp16).
                # The two slots live in different PSUM banks, so the two
                # engines stream concurrently.
                c, pair, g = units[u]
                sA, sB = (2 * u) % 3, (2 * u + 1) % 3
                pa = dyn.tile([128, 1024], f16, tag="pa", bufs=7,
                              name=f"pa_{c}_{pair}_{g}")
                nc.scalar.activation(out=pa, in_=sf3[:, sA, :], func=EXP,
                                     scale=0.125)
                pb = dyn.tile([128, 1024], i16, tag="pb", bufs=7,
                              name=f"pb_{c}_{pair}_{g}")
                nc.vector.tensor_scalar(
                    out=pb, in0=sf3[:, sB, :], scalar1=EXP_A, scalar2=EXP_B,
                    op0=ALU.mult, op1=ALU.add)
                return pa, pb.bitcast(f16)

            def emit_pv(u, pab):
                c, pair, g = units[u]
                for j in range(2):
                    tk = 2 * g + j
                    off = tk * VROW + pair * 192
                    nc.tensor.matmul(
                        out=pv_e, lhsT=va[:, off:off + 128],
                        rhs=pab[j][:, 0:512],
                        start=(tk == 0), stop=(tk == NTT - 1))
                    nc.tensor.matmul(
                        out=pv_o, lhsT=va[:, off + 64:off + 192],
                        rhs=pab[j][:, 512:1024],
                        start=(tk == 0), stop=(tk == NTT - 1))

            def emit_norm(c, pair):
                # pv_e = [O_e; d_e], pv_o = [d_o; O_o]; 1/d via int16
                # magic seed + one fp16 Newton step.  Work is spread:
                # base-aligned O evicts + the magic subtract on ScalarE,
                # cross-base denominator evicts on the DVE.  The Newton
                # chain for pair 1 gates the chunk's O-projection, so it
                # runs on the fast DVE; pair 0's (latency-insensitive)
                # runs on the otherwise-idle GpSimd.
                eng = nc.vector if pair == 1 else nc.gpsimd
                oo = dyn.tile([128, 512], f16, tag="oo", name=f"oo{c}{pair}")
                dd = dyn.tile([128, 512], f16, tag="dd", name=f"dd{c}{pair}")
                nc.scalar.activation(out=oo[0:64, :], in_=pv_e[0:64, :],
                                     func=CPY)
                nc.scalar.activation(out=oo[64:128, :], in_=pv_o[64:128, :],
                                     func=CPY)
                nc.vector.tensor_copy(out=dd[0:64, :], in_=pv_e[64:128, :])
                nc.vector.tensor_copy(out=dd[64:128, :], in_=pv_o[0:64, :])
                r0 = dyn.tile([128, 512], i16, tag="r0", name=f"r0{c}{pair}")
                nc.scalar.activation(out=r0, in_=dd.bitcast(i16), func=CPY,
                                     scale=-1.0, bias=float(MAGIC))
                r = r0.bitcast(f16)
                tn = dyn.tile([128, 512], f16, tag="tn", name=f"tn{c}{pair}")
                eng.tensor_tensor(out=tn, in0=dd, in1=r, op=ALU.mult)
                un = dyn.tile([128, 512], f16, tag="un", name=f"un{c}{pair}")
                eng.tensor_scalar(
                    out=un, in0=tn, scalar1=-1.0, scalar2=2.0,
                    op0=ALU.mult, op1=ALU.add)
                r1 = dyn.tile([128, 512], f16, tag="r1", name=f"r1{c}{pair}")
                eng.tensor_tensor(out=r1, in0=r, in1=un, op=ALU.mult)
                on = dyn.tile([128, 512], f16, tag=f"on{pair}",
                              name=f"on{c}{pair}")
                eng.tensor_tensor(out=on, in0=oo, in1=r1, op=ALU.mult)
                onorm[(c, pair)] = on

            def emit_oproj(c):
                for mt in range(4):
                    for n2 in range(2):
                        j = mt * 2 + n2
                        ops = sf[:, 3072 + (j % 2) * 512:
                                 3072 + (j % 2) * 512 + 512]
                        for pair in range(2):
                            nc.tensor.matmul(
                                out=ops,
                                lhsT=onorm[(c, pair)][:, mt * 128:(mt + 1) * 128],
                                rhs=wo_sb[pair][:, n2 * 512:(n2 + 1) * 512],
                                start=(pair == 0), stop=(pair == 1))
                        osb = dyn.tile([128, 512], f16, tag="osb", bufs=4,
                                       name=f"osb_{c}_{mt}_{n2}")
                        nc.vector.tensor_copy(out=osb, in_=ops)
                        nc.sync.dma_start(
                            out=out[c * 512 + mt * 128:c * 512 + (mt + 1) * 128,
                                    n2 * 512:(n2 + 1) * 512],
                            in_=osb)

            # Emission = per-engine FIFO order.  Skews:
            #  - scores/exp of u+1 before pv(u), so the PE computes the
            #    next unit's scores while ScalarE exp's unit u.
            #  - O-proj(c) occupies banks 6/7 after chunk c's PV; the first
            #    4 PV units of chunk c+1 (same banks) are held back until
            #    O-proj(c) is emitted, hiding the DVE normalization latency
            #    without ever blocking the PE queue head.
            pas = {}
            pv_hold = []
            norm_due = None
            emit_scores(0)
            pas[0] = emit_exp(0)
            for u in range(len(units)):
                if u + 1 < len(units):
                    emit_scores(u + 1)
                    pas[u + 1] = emit_exp(u + 1)
                if norm_due is not None:
                    # skew the normalization one unit late so its ScalarE
                    # ops never block the exp stream at the queue head
                    emit_norm(*norm_due)
                    norm_due = None
                c, pair, g = units[u]
                in_chunk = u % 16
                if c > 0 and in_chunk < 4:
                    pv_hold.append(u)
                    if in_chunk == 3:
                        emit_oproj(c - 1)
                        for uh in pv_hold:
                            emit_pv(uh, pas.pop(uh))
                        pv_hold = []
                else:
                    emit_pv(u, pas.pop(u))
                if g == 7:
                    norm_due = (c, pair)
            emit_norm(*norm_due)
            emit_oproj(NCH - 1)
    if split_waits:
        _split_multi_waits(nc)
    return nc


def _get_nc(split_waits=True):
    key = ("nc", split_waits)
    if key not in _CACHE:
        _CACHE[key] = _build(split_waits)
    return _CACHE[key]


def make_in_maps(x, Wq, bq, Wk, bk, Wv, bv, Wo):
    dt = np.float16
    in_maps = []
    for core in range(8):
        b, g = divmod(core, 4)
        gs = slice(g * G, (g + 1) * G)
        in_maps.append({
            "xT": np.ascontiguousarray(x[b].T).astype(dt),
            "wqT": np.ascontiguousarray(Wq[gs, :].T).astype(dt),
            "wkT": np.ascontiguousarray(Wk[gs, :].T).astype(dt),
            "wvT": np.ascontiguousarray(Wv[gs, :].T).astype(dt),
            "woT": np.ascontiguousarray(Wo[:, gs].T).astype(dt),
            "bqc": np.ascontiguousarray(bq[gs].reshape(2, 128).T).astype(np.float32),
        })
    return in_maps


def host_out_init(bo, bv, Wo):
    """bo + bv @ Wo.T (the bv contribution is exact: softmax rows sum to 1)."""
    return (bo.astype(np.float64)
            + bv.astype(np.float64) @ Wo.T.astype(np.float64)).astype(np.float32)


def kernel(x, Wq, bq, Wk, bk, Wv, bv, Wo, bo):
    from concourse.bass_utils import run_bass_kernel_spmd

    x = np.asarray(x, dtype=np.float32)
    Wq = np.asarray(Wq, dtype=np.float32)
    Wk = np.asarray(Wk, dtype=np.float32)
    Wv = np.asarray(Wv, dtype=np.float32)
    Wo = np.asarray(Wo, dtype=np.float32)
    bq = np.asarray(bq, dtype=np.float32)
    bv = np.asarray(bv, dtype=np.float32)
    bo = np.asarray(bo, dtype=np.float32)

    nc = _get_nc()
    in_maps = make_in_maps(x, Wq, bq, Wk, bk, Wv, bv, Wo)

    res = run_bass_kernel_spmd(nc, in_maps, core_ids=list(range(8)))
    outp = np.tile(host_out_init(bo, bv, Wo)[None, None, :], (2, T, 1))
    for core in range(8):
        outp[core // 4] += res.results[core]["out"].astype(np.float32)
    return outp


# revision 33
# speedup vs baseline: 1.0310x; 1.0128x over previous
"""Multi-head attention Trainium2 Bass kernel (v2).

Problem: B=2, T=2048, D=1024, H=16 heads, dk=64 (fp32).
  out = softmax((x@Wq.T+bq)(x@Wk.T+bk).T / 8) (x@Wv.T+bv) @ Wo.T + bo

Sharding (8 cores): data-parallel over B (2) x tensor-parallel over 4
head-groups of 4 heads.  Core (b, g) computes, for batch b and heads
[4g, 4g+4): Q/K/V projections (column-sliced Wq/Wk/Wv), attention, and
the row-sliced Wo projection, producing a partial (2048, 1024) fp16
output.  Host sums the partials per batch in fp32 and adds the bias
terms.

Bias algebra (removes all device-side bias work except bq):
  - bk shifts every score of a query by a constant -> softmax-invariant
    -> dropped entirely.
  - bv: softmax rows sum to 1, so the bv contribution to the output is
    the constant row bv @ Wo.T -> folded into bo on the host.
  - bq: added on the Q-projection eviction via a per-partition
    tensor_scalar add (Q.T layout has features on partitions).

Per-core device schedule (everything fp16 operands, fp32 PSUM):
  - One persistent PSUM tensor sf [128, 4096] (all 8 banks) managed
    manually with subtile dependency tracking - no pool barriers, so
    the scheduler freely overlaps phases.
  - Projections (k-outer, 8 full-bank chains): K.T -> V -> Q.T, each
    chain accumulates 8 k-tiles; DMAs are issued in consumption order
    so the PE starts as soon as wk0+xt0 land.
  - V stored as V_aug [128, 16*384]: per key-tile, per head-pair block
    [V_even|ones64|V_odd] so the PV matmul also produces the softmax
    denominator (replicated across 64 partitions) for free.
  - Attention per (chunk c of 512 queries, head-pair):  scores.T tiles
    [128 keys, 512q] per head, both heads of the pair packed into one
    1024-wide PSUM slot (row-group-concurrent matmuls, contraction 64).
    3 slots (banks 0-5) rotate; ScalarE exp's TWO slots per ACTIVATE
    (2048 wide, via a 3D AP, negative-stride for the wrap pattern) to
    amortize the ~313-cycle ACT overhead.  PV accumulates in banks 6-7.
  - Normalization: denominators evicted to fp16 SBUF; 1/d via int16
    magic-subtract seed + one fp16 Newton step (beats the DVE's 8
    cycle/element iterative reciprocal ~3x); O * (1/d) in fp16.
  - Output projection accumulates head-pairs in banks 6/7 (after PV is
    evicted), evicts fp16, DMAs fp16 partials out (halves DMA bytes).
"""

import numpy as np

D = 1024          # d_model
T = 2048          # sequence length
G = 256           # features per head-group (4 heads * 64)
DK = 64
NKT = D // 128    # 8 contraction tiles for projections
NTT = T // 128    # 16 key tiles
NCH = T // 512    # 4 query chunks of 512
VROW = 2 * 192    # V_aug row per key tile: 2 blocks of [V_e|ones64|V_o]
MAGIC = 0x7798    # fp16 reciprocal seed: bitcast(MAGIC - bits16(d))
# fp16 Schraudolph exp for the DVE half: bitcast16(rint(s*EXP_A + EXP_B))
# ~= exp(s/8), max rel err ~3% pointwise, ~6.5e-3 end-to-end (softmax
# weights are consistent: the denominator sums the same approximated p).
EXP_A = 0.125 * 1.4426950408889634 * 1024.0
EXP_B = 15360.0 - 44.5

_CACHE = {}


def _split_multi_waits(nc):
    """walrus's TRN2 codegen rejects >1 sync-wait on datapath instruction
    structs.  Hoist every wait of a multi-wait datapath instruction onto
    single-wait NoOps just before it on the same engine queue."""
    import concourse.mybir as mybir

    keep = ("InstEventSemaphore", "InstUnconditionalBranch",
            "InstCall", "InstBranchHint", "InstHalt", "InstNoOp",
            "InstAllEngineBarrier", "InstCompareAndBranch")
    nid = [0]
    for f in nc.m.functions:
        for bb in f.blocks:
            new = []
            for ins in bb.instructions:
                si = ins.sync_info
                waits = list(si.on_wait) if si and si.on_wait else []
                if len(waits) >= 2 and type(ins).__name__ not in keep:
                    for w in waits:
                        nid[0] += 1
                        nop = mybir.InstNoOp(name=f"{ins.name}-wsplit{nid[0]}",
                                             ins=[], outs=[])
                        nop.engine = ins.engine
                        nop.sync_info = mybir.SyncInfo(on_wait=[w], on_update=[])
                        new.append(nop)
                    ins.sync_info = mybir.SyncInfo(
                        on_wait=[], on_update=list(si.on_update or []))
                new.append(ins)
            bb.instructions = new
    return nc


def _build(split_waits=True):
    import concourse.bass as bass
    import concourse.mybir as mybir
    import concourse.tile as tile

    f32 = mybir.dt.float32
    f16 = mybir.dt.float16
    i16 = mybir.dt.int16
    ALU = mybir.AluOpType
    EXP = mybir.ActivationFunctionType.Exp
    CPY = mybir.ActivationFunctionType.Copy
    nc = bass.Bass()

    xT = nc.dram_tensor("xT", [D, T], f16, kind="ExternalInput")
    wqT = nc.dram_tensor("wqT", [D, G], f16, kind="ExternalInput")
    wkT = nc.dram_tensor("wkT", [D, G], f16, kind="ExternalInput")
    wvT = nc.dram_tensor("wvT", [D, G], f16, kind="ExternalInput")
    woT = nc.dram_tensor("woT", [G, D], f16, kind="ExternalInput")
    bqc = nc.dram_tensor("bqc", [128, 2], f32, kind="ExternalInput")
    out = nc.dram_tensor("out", [T, D], f16, kind="ExternalOutput")

    with tile.TileContext(nc) as tc:
        with tc.tile_pool(name="sb", bufs=1) as sb, \
             tc.tile_pool(name="dyn", bufs=2) as dyn, \
             tc.tile_pool(name="ps", bufs=1, space="PSUM") as ps:

            # ---- DMAs in consumption order ----
            wk_sb, xt = [], []
            for k in range(NKT):
                t = sb.tile([128, G], f16, tag=f"wk{k}", name=f"wk{k}")
                nc.sync.dma_start(out=t, in_=wkT[k * 128:(k + 1) * 128, :])
                wk_sb.append(t)
                t = sb.tile([128, T], f16, tag=f"xt{k}", name=f"xt{k}")
                nc.sync.dma_start(out=t, in_=xT[k * 128:(k + 1) * 128, :])
                xt.append(t)
            bq_sb = sb.tile([128, 2], f32, tag="bq", name="bq_sb")
            nc.sync.dma_start(out=bq_sb, in_=bqc[:, :])
            # warm the ScalarE exp table-set (~2.7us) during the DMA wait
            scr = sb.tile([128, 2], f16, tag="scr", name="scr")
            nc.scalar.activation(out=scr, in_=bq_sb, func=EXP, scale=0.0)
            wv_sb, wq_sb = [], []
            for nm, dram, lst in (("wv", wvT, wv_sb), ("wq", wqT, wq_sb)):
                for k in range(NKT):
                    t = sb.tile([128, G], f16, tag=f"{nm}{k}", name=f"{nm}{k}")
                    nc.sync.dma_start(out=t, in_=dram[k * 128:(k + 1) * 128, :])
                    lst.append(t)
            wo_sb = []
            for p2 in range(2):
                t = sb.tile([128, D], f16, tag=f"wo{p2}", name=f"wo{p2}")
                nc.sync.dma_start(out=t, in_=woT[p2 * 128:(p2 + 1) * 128, :])
                wo_sb.append(t)

            # ---- persistent SBUF ----
            qt = [sb.tile([128, T], f16, tag=f"qt{p}", name=f"qt{p}")
                  for p in range(2)]
            kt = [sb.tile([128, T], f16, tag=f"kt{p}", name=f"kt{p}")
                  for p in range(2)]
            va = sb.tile([128, NTT * VROW], f16, tag="va", name="va")
            va6 = va.rearrange("p (t b x) -> p t b x", t=NTT, b=6)
            nc.vector.memset(va6[:, :, 1::3, :], 1.0)   # ones64 columns

            # ---- the one PSUM tensor: 8 banks, manual ranges ----
            sf = ps.tile([128, 4096], f32, tag="sf", name="sf")
            sf3 = sf[:, 0:3072].rearrange("p (s x) -> p s x", s=3)
            pv_e = sf[:, 3072:3584]
            pv_o = sf[:, 3584:4096]

            def chain(i):       # 8 full-bank projection chains
                return sf[:, i * 512:(i + 1) * 512]

            # ---- K.T projection: chains (p2, c), k-outer ----
            for k in range(NKT):
                for i in range(8):
                    p2, c = divmod(i, 4)
                    nc.tensor.matmul(
                        out=chain(i),
                        lhsT=wk_sb[k][:, p2 * 128:(p2 + 1) * 128],
                        rhs=xt[k][:, c * 512:(c + 1) * 512],
                        start=(k == 0), stop=(k == NKT - 1))
            for i in range(8):
                p2, c = divmod(i, 4)
                nc.vector.tensor_copy(
                    out=kt[p2][:, c * 512:(c + 1) * 512], in_=chain(i))

            # ---- V projection: chain-major so V tiles complete (and are
            # evicted) progressively; bank ring tt%8 handles wave reuse ----
            va5 = va.rearrange("p (t pr b x) -> p t pr b x", t=NTT, pr=2, b=3)
            for tt in range(NTT):
                base = (tt % 8) * 512
                for k in range(NKT):
                    nc.tensor.matmul(
                        out=sf[:, base:base + G],
                        lhsT=xt[k][:, tt * 128:(tt + 1) * 128],
                        rhs=wv_sb[k][:, :],
                        start=(k == 0), stop=(k == NKT - 1))
                nc.vector.tensor_copy(
                    out=va5[:, tt, :, 0::2, :],
                    in_=sf[:, base:base + 256].rearrange(
                        "p (pr h x) -> p pr h x", pr=2, h=2))

            # ---- Q.T projection (+bq on eviction), chain-major, chunk-0
            # chains first so stage B's first scores unblock earliest ----
            for i, (c, p2) in enumerate((c, p2) for c in range(4)
                                        for p2 in range(2)):
                for k in range(NKT):
                    nc.tensor.matmul(
                        out=chain(i),
                        lhsT=wq_sb[k][:, p2 * 128:(p2 + 1) * 128],
                        rhs=xt[k][:, c * 512:(c + 1) * 512],
                        start=(k == 0), stop=(k == NKT - 1))
                nc.vector.tensor_scalar(
                    out=qt[p2][:, c * 512:(c + 1) * 512], in0=chain(i),
                    scalar1=bq_sb[:, p2:p2 + 1], scalar2=None, op0=ALU.add)

            # ---- attention + output projection ----
            # Flat software pipeline over 64 "units" (one unit = 2 key
            # tiles of one (chunk, head-pair)).  Per-engine queues are
            # strict FIFO, so emission order IS the schedule skeleton:
            # scores(u+1) must be emitted BEFORE pv(u) (which waits on
            # exp(u)) or the PE queue head blocks and the whole loop
            # serializes; O-proj is emitted 2 units after its chunk ends
            # so the DVE normalization latency is hidden.
            units = [(c, pair, g) for c in range(NCH) for pair in range(2)
                     for g in range(8)]
            onorm = {}          # (c, pair) -> normalized O tile

            def emit_scores(u):
                c, pair, g = units[u]
                cs = slice(c * 512, (c + 1) * 512)
                sA, sB = (2 * u) % 3, (2 * u + 1) % 3
                for tk, s in ((2 * g, sA), (2 * g + 1, sB)):
                    for h in range(2):   # packed row-group pair
                        nc.tensor.matmul(
                            out=sf[:, s * 1024 + h * 512:
                                   s * 1024 + (h + 1) * 512],
                            lhsT=kt[pair][h * 64:(h + 1) * 64,
                                          tk * 128:(tk + 1) * 128],
                            rhs=qt[pair][h * 64:(h + 1) * 64, cs],
                            start=True, stop=True)

            def emit_exp(u):
                # tkA: exact exp on ScalarE; tkB: Schraudolph fast-exp on
                # the DVE (one fused mul-add into int16, bitcast to fp16).
                # The two slots live in different PSUM banks, so the two
                # engines stream concurrently.
                c, pair, g = units[u]
                sA, sB = (2 * u) % 3, (2 * u + 1) % 3
                pa = dyn.tile([128, 1024], f16, tag="pa", bufs=9,
                              name=f"pa_{c}_{pair}_{g}")
                nc.scalar.activation(out=pa, in_=sf3[:, sA, :], func=EXP,
                                     scale=0.125)
                pb = dyn.tile([128, 1024], i16, tag="pb", bufs=9,
                              name=f"pb_{c}_{pair}_{g}")
                nc.vector.tensor_scalar(
                    out=pb, in0=sf3[:, sB, :], scalar1=EXP_A, scalar2=EXP_B,
                    op0=ALU.mult, op1=ALU.add)
                return pa, pb.bitcast(f16)

            def emit_pv(u, pab):
                c, pair, g = units[u]
                for j in range(2):
                    tk = 2 * g + j
                    off = tk * VROW + pair * 192
                    nc.tensor.matmul(
                        out=pv_e, lhsT=va[:, off:off + 128],
                        rhs=pab[j][:, 0:512],
                        start=(tk == 0), stop=(tk == NTT - 1))
                    nc.tensor.matmul(
                        out=pv_o, lhsT=va[:, off + 64:off + 192],
                        rhs=pab[j][:, 512:1024],
                        start=(tk == 0), stop=(tk == NTT - 1))

            def emit_norm(c, pair):
                # pv_e = [O_e; d_e], pv_o = [d_o; O_o]; 1/d via int16
                # magic seed + one fp16 Newton step.  Work is spread:
                # base-aligned O evicts + the magic subtract on ScalarE,
                # cross-base denominator evicts on the DVE.  The Newton
                # chain for pair 1 gates the chunk's O-projection, so it
                # runs on the fast DVE; pair 0's (latency-insensitive)
                # runs on the otherwise-idle GpSimd.
                eng = nc.vector if pair == 1 else nc.gpsimd
                oo = dyn.tile([128, 512], f16, tag="oo", name=f"oo{c}{pair}")
                dd = dyn.tile([128, 512], f16, tag="dd", name=f"dd{c}{pair}")
                nc.scalar.activation(out=oo[0:64, :], in_=pv_e[0:64, :],
                                     func=CPY)
                nc.scalar.activation(out=oo[64:128, :], in_=pv_o[64:128, :],
                                     func=CPY)
                nc.vector.tensor_copy(out=dd[0:64, :], in_=pv_e[64:128, :])
                nc.vector.tensor_copy(out=dd[64:128, :], in_=pv_o[0:64, :])
                r0 = dyn.tile([128, 512], i16, tag="r0", name=f"r0{c}{pair}")
                nc.scalar.activation(out=r0, in_=dd.bitcast(i16), func=CPY,
                                     scale=-1.0, bias=float(MAGIC))
                r = r0.bitcast(f16)
                tn = dyn.tile([128, 512], f16, tag="tn", name=f"tn{c}{pair}")
                eng.tensor_tensor(out=tn, in0=dd, in1=r, op=ALU.mult)
                un = dyn.tile([128, 512], f16, tag="un", name=f"un{c}{pair}")
                eng.tensor_scalar(
                    out=un, in0=tn, scalar1=-1.0, scalar2=2.0,
                    op0=ALU.mult, op1=ALU.add)
                r1 = dyn.tile([128, 512], f16, tag="r1", name=f"r1{c}{pair}")
                eng.tensor_tensor(out=r1, in0=r, in1=un, op=ALU.mult)
                on = dyn.tile([128, 512], f16, tag=f"on{pair}",
                              name=f"on{c}{pair}")
                eng.tensor_tensor(out=on, in0=oo, in1=r1, op=ALU.mult)
                onorm[(c, pair)] = on

            def emit_oproj(c):
                for mt in range(4):
                    for n2 in range(2):
                        j = mt * 2 + n2
                        ops = sf[:, 3072 + (j % 2) * 512:
                                 3072 + (j % 2) * 512 + 512]
                        for pair in range(2):
                            nc.tensor.matmul(
                                out=ops,
                                lhsT=onorm[(c, pair)][:, mt * 128:(mt + 1) * 128],
                                rhs=wo_sb[pair][:, n2 * 512:(n2 + 1) * 512],
                                start=(pair == 0), stop=(pair == 1))
                        osb = dyn.tile([128, 512], f16, tag="osb", bufs=4,
                                       name=f"osb_{c}_{mt}_{n2}")
                        nc.vector.tensor_copy(out=osb, in_=ops)
                        nc.sync.dma_start(
                            out=out[c * 512 + mt * 128:c * 512 + (mt + 1) * 128,
                                    n2 * 512:(n2 + 1) * 512],
                            in_=osb)

            # Emission = per-engine FIFO order.  Skews:
            #  - scores/exp of u+1 before pv(u), so the PE computes the
            #    next unit's scores while ScalarE exp's unit u.
            #  - O-proj(c) occupies banks 6/7 after chunk c's PV; the first
            #    4 PV units of chunk c+1 (same banks) are held back until
            #    O-proj(c) is emitted, hiding the DVE normalization latency
            #    without ever blocking the PE queue head.
            pas = {}
            pv_hold = []
            norm_due = None
            emit_scores(0)
            pas[0] = emit_exp(0)
            for u in range(len(units)):
                if u + 1 < len(units):
                    emit_scores(u + 1)
                    pas[u + 1] = emit_exp(u + 1)
                if norm_due is not None:
                    # skew the normalization one unit late so its ScalarE
                    # ops never block the exp stream at the queue head
                    emit_norm(*norm_due)
                    norm_due = None
                c, pair, g = units[u]
                in_chunk = u % 16
                if c > 0 and in_chunk < 4:
                    pv_hold.append(u)
                    if in_chunk == 3:
                        emit_oproj(c - 1)
                        for uh in pv_hold:
                            emit_pv(uh, pas.pop(uh))
                        pv_hold = []
                else:
                    emit_pv(u, pas.pop(u))
                if g == 7:
                    norm_due = (c, pair)
            emit_norm(*norm_due)
            emit_oproj(NCH - 1)
    if split_waits:
        _split_multi_waits(nc)
    return nc


def _get_nc(split_waits=True):
    key = ("nc", split_waits)
    if key not in _CACHE:
        _CACHE[key] = _build(split_waits)
    return _CACHE[key]


def make_in_maps(x, Wq, bq, Wk, bk, Wv, bv, Wo):
    dt = np.float16
    in_maps = []
    for core in range(8):
        b, g = divmod(core, 4)
        gs = slice(g * G, (g + 1) * G)
        in_maps.append({
            "xT": np.ascontiguousarray(x[b].T).astype(dt),
            "wqT": np.ascontiguousarray(Wq[gs, :].T).astype(dt),
            "wkT": np.ascontiguousarray(Wk[gs, :].T).astype(dt),
            "wvT": np.ascontiguousarray(Wv[gs, :].T).astype(dt),
            "woT": np.ascontiguousarray(Wo[:, gs].T).astype(dt),
            "bqc": np.ascontiguousarray(bq[gs].reshape(2, 128).T).astype(np.float32),
        })
    return in_maps


def host_out_init(bo, bv, Wo):
    """bo + bv @ Wo.T (the bv contribution is exact: softmax rows sum to 1)."""
    return (bo.astype(np.float64)
            + bv.astype(np.float64) @ Wo.T.astype(np.float64)).astype(np.float32)


def kernel(x, Wq, bq, Wk, bk, Wv, bv, Wo, bo):
    from concourse.bass_utils import run_bass_kernel_spmd

    x = np.asarray(x, dtype=np.float32)
    Wq = np.asarray(Wq, dtype=np.float32)
    Wk = np.asarray(Wk, dtype=np.float32)
    Wv = np.asarray(Wv, dtype=np.float32)
    Wo = np.asarray(Wo, dtype=np.float32)
    bq = np.asarray(bq, dtype=np.float32)
    bv = np.asarray(bv, dtype=np.float32)
    bo = np.asarray(bo, dtype=np.float32)

    nc = _get_nc()
    in_maps = make_in_maps(x, Wq, bq, Wk, bk, Wv, bv, Wo)

    res = run_bass_kernel_spmd(nc, in_maps, core_ids=list(range(8)))
    outp = np.tile(host_out_init(bo, bv, Wo)[None, None, :], (2, T, 1))
    for core in range(8):
        outp[core // 4] += res.results[core]["out"].astype(np.float32)
    return outp
